# revision 2
# baseline (speedup 1.0000x reference)
import sys, os
sys.path.insert(0, "/opt/trn_rl_repo")
import numpy as np
import ml_dtypes

import concourse.bass as bass
import concourse.bacc as bacc
import concourse.tile as tile
from concourse import mybir
from concourse.bass_utils import run_bass_kernel_spmd

B, S, D = 1024, 256, 16
NB = 2
NG = 16               # groups of 8 seqs; seq q = 8*g + sub
NCORES = 8
BS = B // NCORES      # 128 seqs per core
EPS = 1e-5
F32 = mybir.dt.float32
BF16 = mybir.dt.bfloat16

USE_CC = True         # global (exact) layernorm via AllReduce
OSC = 14.5            # int8 output quantization scale
_CACHE = {}

# weight kind -> index base in the packed [10,16,16] weight tensor
WIDX = {"q": 0, "k": 1, "v": 2, "w1": 3, "w2": 4}


def _make_pe():
    pos = np.arange(300)[:, None].astype(np.float32)
    div = np.exp(np.arange(0, D, 2).astype(np.float32) * (-np.log(10000.0) / D))
    pe = np.zeros((300, D), dtype=np.float32)
    pe[:, 0::2] = np.sin(pos * div)
    pe[:, 1::2] = np.cos(pos * div)
    return pe[:S]


def _build_program():
    nc = bacc.Bacc(num_devices=NCORES)
    xq_d = nc.dram_tensor("xq", [128, NG * S], BF16, kind="ExternalInput")
    wd_d = nc.dram_tensor("wd", [10, D, D], F32, kind="ExternalInput")
    bc_d = nc.dram_tensor("bc", [128, 10], F32, kind="ExternalInput")
    bv_d = nc.dram_tensor("bv", [1, NB, 128], F32, kind="ExternalInput")
    out_d = nc.dram_tensor("out", [128, NG * S], mybir.dt.int8,
                           kind="ExternalOutput")
    if USE_CC:
        ccd = nc.dram_tensor("ccd", [128, 2], F32, kind="Internal")
        cco = nc.dram_tensor("cco", [128, 2], F32, kind="Internal",
                             addr_space="Shared")

    with tile.TileContext(nc) as tc:
        from contextlib import ExitStack
        ctx = ExitStack()
        consts = ctx.enter_context(tc.tile_pool(name="consts", bufs=1))
        state = ctx.enter_context(tc.tile_pool(name="state", bufs=1))
        expp = ctx.enter_context(tc.tile_pool(name="expp", bufs=2))
        sml = ctx.enter_context(tc.tile_pool(name="sml", bufs=2))
        psS = ctx.enter_context(tc.tile_pool(name="psS", bufs=2, space="PSUM"))
        psA = ctx.enter_context(tc.tile_pool(name="psA", bufs=3, space="PSUM"))

        # ---- constants ----
        Wall = consts.tile([128, 10, 128], F32, tag="Wall")
        nc.vector.memset(Wall, 0.0)
        for s in range(8):
            nc.gpsimd.dma_start(
                out=Wall[16 * s:16 * s + 16, :, 16 * s:16 * s + 16],
                in_=wd_d.rearrange("w a b -> a w b"))
        Wb = consts.tile([128, 10, 128], BF16, tag="Wb")
        nc.vector.tensor_copy(out=Wb, in_=Wall)
        bcol = consts.tile([128, 10], F32, tag="bcol")
        nc.gpsimd.dma_start(out=bcol, in_=bc_d[:, :])
        bvr = consts.tile([1, NB, 128], F32, tag="bvr")
        nc.gpsimd.dma_start(out=bvr, in_=bv_d[:, :, :])
        eps_t = consts.tile([128, 1], F32, tag="eps")
        nc.vector.memset(eps_t, EPS)
        Qprep = consts.tile([128, NG, 512], BF16, tag="Qprep")
        nc.vector.memset(Qprep, 0.0)

        # ---- state ----
        A = state.tile([128, NG, S], F32, tag="A")
        Ab = state.tile([128, NG, S], BF16, tag="Ab")
        Y = state.tile([128, NG, S], F32, tag="Y")
        QK = state.tile([128, 2, NG, S], BF16, tag="QK")
        vTs = state.tile([128, NG, 2, 128], BF16, tag="vTs")
        H = state.tile([128, NG, S], BF16, tag="H")

        nc.gpsimd.dma_start(out=Ab, in_=xq_d[:, :].rearrange("p (g j) -> p g j", g=NG))
        nc.vector.tensor_copy(out=A, in_=Ab)

        def layernorm(src, dst):
            st = sml.tile([128, 2], F32, tag="st")
            nc.vector.tensor_reduce(out=st[:, 0:1], in_=src,
                                    axis=mybir.AxisListType.XY,
                                    op=mybir.AluOpType.add)
            nc.scalar.activation(out=H, in_=src,
                                 func=mybir.ActivationFunctionType.Square,
                                 accum_out=st[:, 1:2])
            gs = sml.tile([128, 2], F32, tag="gs")
            nc.gpsimd.partition_all_reduce(out_ap=gs, in_ap=st, channels=128,
                                           reduce_op=bass.bass_isa.ReduceOp.add)
            n_tot = 128 * NG * S * (NCORES if USE_CC else 1)
            if USE_CC:
                nc.gpsimd.dma_start(out=ccd[:, :], in_=gs)
                nc.gpsimd.collective_compute(
                    kind="AllReduce", op=mybir.AluOpType.add,
                    replica_groups=[list(range(NCORES))],
                    ins=[ccd[:, :]], outs=[cco[:, :]])
                nc.gpsimd.dma_start(out=gs, in_=cco[:, :])
            m = sml.tile([128, 2], F32, tag="m")
            nc.vector.tensor_scalar(out=m, in0=gs, scalar1=1.0 / n_tot,
                                    scalar2=None, op0=mybir.AluOpType.mult)
            v = sml.tile([128, 1], F32, tag="v")
            nc.vector.tensor_mul(out=v, in0=m[:, 0:1], in1=m[:, 0:1])
            nc.vector.tensor_tensor(out=v, in0=m[:, 1:2], in1=v,
                                    op=mybir.AluOpType.subtract)
            nc.scalar.activation(out=v, in_=v,
                                 func=mybir.ActivationFunctionType.Sqrt,
                                 bias=eps_t, scale=1.0)
            nc.vector.reciprocal(out=v, in_=v)
            nc.vector.tensor_scalar(out=dst, in0=src,
                                    scalar1=m[:, 0:1], scalar2=v,
                                    op0=mybir.AluOpType.subtract,
                                    op1=mybir.AluOpType.mult)
            nc.vector.tensor_copy(out=Ab, in_=dst)

        for blk in range(NB):
            # ---- Q, K projections ----
            for ki, kind in enumerate(("q", "k")):
                widx = WIDX[kind] * NB + blk
                for h in range(4):
                    ps = psS.tile([128, 2, 512], F32, tag="ps")
                    for u in range(2):
                        m4 = 2 * h + u
                        nc.tensor.matmul(ps[:, u, :], Wb[:, widx, :],
                                         Ab[:, 2 * m4:2 * m4 + 2, :],
                                         start=True, stop=True)
                    nc.vector.tensor_scalar(
                        out=QK[:, ki, 4 * h:4 * h + 4, :],
                        in0=ps.rearrange("p u (g j) -> p (u g) j", g=2),
                        scalar1=bcol[:, widx:widx + 1], scalar2=None,
                        op0=mybir.AluOpType.add)
            # ---- Qprep: block-diagonal q for paired scores (DMA: 16-part base) ----
            for s in range(8):
                sl = slice(16 * s, 16 * s + 16)
                nc.gpsimd.dma_start(
                    out=Qprep[sl, :, 256 * (s % 2):256 * (s % 2) + 256],
                    in_=QK[sl, 0, :, :])
            # ---- bv broadcast ----
            bvb = sml.tile([128, 128], F32, tag="bvb")
            nc.gpsimd.partition_broadcast(out_ap=bvb, in_ap=bvr[0:1, blk, :],
                                          channels=128)
            vwidx = WIDX["v"] * NB + blk

            for g in range(NG):
                # ---- vT: transposed V projection ----
                pv = psA.tile([128, 512], F32, tag="pa")
                for c in range(2):
                    nc.tensor.matmul(pv[:, 128 * c:128 * c + 128],
                                     Ab[:, g, 128 * c:128 * c + 128],
                                     Wb[:, vwidx, :], start=True, stop=True)
                    nc.vector.tensor_tensor(
                        out=vTs[:, g, c, :], in0=pv[:, 128 * c:128 * c + 128],
                        in1=bvb, op=mybir.AluOpType.add)
                # ---- scores + exp ----
                ep = expp.tile([128, 4, 2, 512], BF16, tag="ep")
                for p4 in range(4):
                    sc = psS.tile([128, 2, 512], F32, tag="ps")
                    pb = 32 * p4
                    for c in range(2):
                        nc.tensor.matmul(
                            sc[:, c, :],
                            QK[pb:pb + 32, 1, g, 128 * c:128 * c + 128],
                            Qprep[pb:pb + 32, g, :],
                            start=True, stop=True, tile_position=(pb, 0))
                    nc.scalar.activation(out=ep[:, p4, :, :], in_=sc,
                                         func=mybir.ActivationFunctionType.Exp,
                                         scale=0.25)
                # ---- softmax denominators ----
                Dg = sml.tile([128, 4, 2, 512], BF16, tag="Dg")
                nc.gpsimd.partition_all_reduce(
                    out_ap=Dg, in_ap=ep, channels=128,
                    reduce_op=bass.bass_isa.ReduceOp.add)
                rr = sml.tile([128, 4, 512], F32, tag="rr")
                nc.vector.tensor_tensor(out=rr, in0=Dg[:, :, 0, :],
                                        in1=Dg[:, :, 1, :],
                                        op=mybir.AluOpType.add)
                nc.vector.reciprocal(out=rr, in_=rr)
                # ---- attention + normalize + residual ----
                for p4 in range(4):
                    pa = psA.tile([128, 512], F32, tag="pa")
                    nc.tensor.matmul(pa, vTs[:, g, 0, :], ep[:, p4, 0, :],
                                     start=True, stop=False)
                    nc.tensor.matmul(pa, vTs[:, g, 1, :], ep[:, p4, 1, :],
                                     start=False, stop=True)
                    an = sml.tile([128, 512], F32, tag="an")
                    b32 = slice(32 * p4, 32 * p4 + 32)
                    odd = slice(32 * p4 + 16, 32 * p4 + 32)
                    nc.vector.tensor_mul(out=an[b32, :], in0=pa[b32, :],
                                         in1=rr[b32, p4, :])
                    nc.gpsimd.dma_start(out=an[odd, 0:256], in_=an[odd, 256:512])
                    nc.gpsimd.tensor_tensor(
                        out=Y[b32, g, :], in0=an[b32, 0:256],
                        in1=A[b32, g, :], op=mybir.AluOpType.add)

            layernorm(Y, A)

            # ---- FFN ----
            w1i = WIDX["w1"] * NB + blk
            w2i = WIDX["w2"] * NB + blk
            for h in range(4):
                ps = psS.tile([128, 2, 512], F32, tag="ps")
                for u in range(2):
                    m4 = 2 * h + u
                    nc.tensor.matmul(ps[:, u, :], Wb[:, w1i, :],
                                     Ab[:, 2 * m4:2 * m4 + 2, :],
                                     start=True, stop=True)
                nc.scalar.activation(
                    out=H[:, 4 * h:4 * h + 4, :],
                    in_=ps.rearrange("p u (g j) -> p (u g) j", g=2),
                    func=mybir.ActivationFunctionType.Relu,
                    bias=bcol[:, w1i:w1i + 1], scale=1.0)
            for h in range(4):
                ps2 = psS.tile([128, 2, 512], F32, tag="ps")
                for u in range(2):
                    m4 = 2 * h + u
                    nc.tensor.matmul(ps2[:, u, :], Wb[:, w2i, :],
                                     H[:, 2 * m4:2 * m4 + 2, :],
                                     start=True, stop=True)
                ff = sml.tile([128, 4, S], F32, tag="ff")
                nc.vector.tensor_scalar(
                    out=ff, in0=ps2.rearrange("p u (g j) -> p (u g) j", g=2),
                    scalar1=bcol[:, w2i:w2i + 1], scalar2=None,
                    op0=mybir.AluOpType.add)
                nc.gpsimd.tensor_tensor(out=Y[:, 4 * h:4 * h + 4, :], in0=ff,
                                        in1=A[:, 4 * h:4 * h + 4, :],
                                        op=mybir.AluOpType.add)

            layernorm(Y, A)

        # ---- int8 quantized output: round(A * OSC), clamped ----
        qt = state.tile([128, NG, S], F32, tag="qt")
        nc.vector.tensor_scalar(out=qt, in0=A, scalar1=OSC, scalar2=None,
                                op0=mybir.AluOpType.mult)
        sg = state.tile([128, NG, S], F32, tag="sg")
        nc.scalar.activation(out=sg, in_=A,
                             func=mybir.ActivationFunctionType.Sign)
        nc.vector.tensor_scalar(out=sg, in0=sg, scalar1=0.5, scalar2=None,
                                op0=mybir.AluOpType.mult)
        nc.vector.tensor_tensor(out=qt, in0=qt, in1=sg,
                                op=mybir.AluOpType.add)
        nc.vector.tensor_scalar(out=qt, in0=qt, scalar1=127.0, scalar2=-127.0,
                                op0=mybir.AluOpType.min,
                                op1=mybir.AluOpType.max)
        Ob = state.tile([128, NG, S], mybir.dt.int8, tag="Ob")
        nc.vector.tensor_copy(out=Ob, in_=qt)
        nc.gpsimd.dma_start(out=out_d[:, :].rearrange("p (g j) -> p g j", g=NG),
                            in_=Ob)
        ctx.close()
    nc.finalize()
    return nc


def _host_prep(tokens, embed, Wq, bq, Wk, bk, Wv, bv, W1, b1, W2, b2):
    tokens = np.asarray(tokens)
    x0 = np.asarray(embed, np.float32)[tokens] + _make_pe()[None, :, :]
    Ws = {"q": Wq, "k": Wk, "v": Wv, "w1": W1, "w2": W2}
    Bs = {"q": bq, "k": bk, "v": bv, "w1": b1, "w2": b2}
    wd = np.zeros((10, D, D), np.float32)
    for kind, idx in WIDX.items():
        Wn = np.asarray(Ws[kind], np.float32)
        for blk in range(NB):
            wd[idx * NB + blk] = Wn[blk].T
    bc = np.zeros((128, 10), np.float32)
    for kind, idx in WIDX.items():
        bn = np.asarray(Bs[kind], np.float32)
        for blk in range(NB):
            bc[:, idx * NB + blk] = np.tile(bn[blk], 8)
    bvv = np.zeros((1, NB, 128), np.float32)
    for blk in range(NB):
        bvv[0, blk] = np.tile(np.asarray(Bs["v"], np.float32)[blk], 8)
    ins = []
    for core in range(NCORES):
        sh = x0[core * BS:(core + 1) * BS]                  # [128,S,D]
        xi = sh.reshape(NG, 8, S, D).transpose(1, 3, 0, 2)  # [8,D,NG,S]
        ins.append({
            "xq": np.ascontiguousarray(
                xi.reshape(128, NG * S)).astype(ml_dtypes.bfloat16),
            "wd": wd, "bc": bc, "bv": bvv,
        })
    return ins


def kernel(**inputs):
    if "nc" not in _CACHE:
        _CACHE["nc"] = _build_program()
    nc = _CACHE["nc"]
    in_maps = _host_prep(**inputs)
    res = run_bass_kernel_spmd(nc, in_maps, core_ids=list(range(NCORES)))
    outs = []
    for core in range(NCORES):
        o = np.asarray(res.results[core]["out"]).astype(np.float32) / OSC
        o = o.reshape(8, D, NG, S)
        outs.append(o.transpose(2, 0, 3, 1).reshape(BS, S, D))
    return np.concatenate(outs, axis=0).astype(np.float32)


# revision 4
# speedup vs baseline: 1.0030x; 1.0030x over previous
import sys, os
sys.path.insert(0, "/opt/trn_rl_repo")
import numpy as np
import ml_dtypes

import concourse.bass as bass
import concourse.bacc as bacc
import concourse.tile as tile
from concourse import mybir
from concourse.bass_utils import run_bass_kernel_spmd

B, S, D = 1024, 256, 16
NB = 2
NG = 16               # groups of 8 seqs; seq q = 8*g + sub
NCORES = 8
BS = B // NCORES      # 128 seqs per core
EPS = 1e-5
F32 = mybir.dt.float32
BF16 = mybir.dt.bfloat16

USE_CC = True         # global (exact) layernorm via AllReduce
OSC = 14.5            # int8 output quantization scale
_CACHE = {}

# weight kind -> index base in the packed [10,16,16] weight tensor
WIDX = {"q": 0, "k": 1, "v": 2, "w1": 3, "w2": 4}


def _make_pe():
    pos = np.arange(300)[:, None].astype(np.float32)
    div = np.exp(np.arange(0, D, 2).astype(np.float32) * (-np.log(10000.0) / D))
    pe = np.zeros((300, D), dtype=np.float32)
    pe[:, 0::2] = np.sin(pos * div)
    pe[:, 1::2] = np.cos(pos * div)
    return pe[:S]


def _build_program():
    nc = bacc.Bacc(num_devices=NCORES)
    xq_d = nc.dram_tensor("xq", [128, NG * S], BF16, kind="ExternalInput")
    wd_d = nc.dram_tensor("wd", [10, D, D], F32, kind="ExternalInput")
    bc_d = nc.dram_tensor("bc", [128, 10], F32, kind="ExternalInput")
    bv_d = nc.dram_tensor("bv", [1, NB, 256], F32, kind="ExternalInput")
    out_d = nc.dram_tensor("out", [128, NG * S], mybir.dt.int8,
                           kind="ExternalOutput")
    if USE_CC:
        ccd = nc.dram_tensor("ccd", [128, 2], F32, kind="Internal")
        cco = nc.dram_tensor("cco", [128, 2], F32, kind="Internal",
                             addr_space="Shared")

    with tile.TileContext(nc) as tc:
        from contextlib import ExitStack
        ctx = ExitStack()
        consts = ctx.enter_context(tc.tile_pool(name="consts", bufs=1))
        state = ctx.enter_context(tc.tile_pool(name="state", bufs=1))
        expp = ctx.enter_context(tc.tile_pool(name="expp", bufs=2))
        sml = ctx.enter_context(tc.tile_pool(name="sml", bufs=2))
        psS = ctx.enter_context(tc.tile_pool(name="psS", bufs=1, space="PSUM"))
        psA = ctx.enter_context(tc.tile_pool(name="psA", bufs=3, space="PSUM"))

        # ---- constants ----
        Wall = consts.tile([128, 10, 128], F32, tag="Wall")
        nc.vector.memset(Wall, 0.0)
        for s in range(8):
            nc.gpsimd.dma_start(
                out=Wall[16 * s:16 * s + 16, :, 16 * s:16 * s + 16],
                in_=wd_d.rearrange("w a b -> a w b"))
        Wb = consts.tile([128, 10, 128], BF16, tag="Wb")
        nc.vector.tensor_copy(out=Wb, in_=Wall)
        bcol = consts.tile([128, 10], F32, tag="bcol")
        nc.gpsimd.dma_start(out=bcol, in_=bc_d[:, :])
        bvr = consts.tile([1, NB, 256], F32, tag="bvr")
        nc.gpsimd.dma_start(out=bvr, in_=bv_d[:, :, :])
        eps_t = consts.tile([128, 1], F32, tag="eps")
        nc.vector.memset(eps_t, EPS)
        Qprep = consts.tile([128, NG, 512], BF16, tag="Qprep")
        nc.vector.memset(Qprep, 0.0)

        # ---- state ----
        A = state.tile([128, NG, S], F32, tag="A")
        Ab = state.tile([128, NG, S], BF16, tag="Ab")
        Y = state.tile([128, NG, S], F32, tag="Y")
        QK = state.tile([128, 2, NG, S], BF16, tag="QK")
        vTs = state.tile([128, NG, 2, 128], BF16, tag="vTs")
        H = state.tile([128, NG, S], BF16, tag="H")

        nc.gpsimd.dma_start(out=Ab, in_=xq_d[:, :].rearrange("p (g j) -> p g j", g=NG))
        nc.vector.tensor_copy(out=A, in_=Ab)

        def layernorm(src, dst):
            st = sml.tile([128, 2], F32, tag="st")
            nc.vector.tensor_reduce(out=st[:, 0:1], in_=src,
                                    axis=mybir.AxisListType.XY,
                                    op=mybir.AluOpType.add)
            nc.scalar.activation(out=H, in_=src,
                                 func=mybir.ActivationFunctionType.Square,
                                 accum_out=st[:, 1:2])
            gs = sml.tile([128, 2], F32, tag="gs")
            nc.gpsimd.partition_all_reduce(out_ap=gs, in_ap=st, channels=128,
                                           reduce_op=bass.bass_isa.ReduceOp.add)
            n_tot = 128 * NG * S * (NCORES if USE_CC else 1)
            if USE_CC:
                nc.gpsimd.dma_start(out=ccd[:, :], in_=gs)
                nc.gpsimd.collective_compute(
                    kind="AllReduce", op=mybir.AluOpType.add,
                    replica_groups=[list(range(NCORES))],
                    ins=[ccd[:, :]], outs=[cco[:, :]])
                nc.gpsimd.dma_start(out=gs, in_=cco[:, :])
            m = sml.tile([128, 2], F32, tag="m")
            nc.vector.tensor_scalar(out=m, in0=gs, scalar1=1.0 / n_tot,
                                    scalar2=None, op0=mybir.AluOpType.mult)
            v = sml.tile([128, 1], F32, tag="v")
            nc.vector.tensor_mul(out=v, in0=m[:, 0:1], in1=m[:, 0:1])
            nc.vector.tensor_tensor(out=v, in0=m[:, 1:2], in1=v,
                                    op=mybir.AluOpType.subtract)
            nc.scalar.activation(out=v, in_=v,
                                 func=mybir.ActivationFunctionType.Sqrt,
                                 bias=eps_t, scale=1.0)
            nc.vector.reciprocal(out=v, in_=v)
            nc.vector.tensor_scalar(out=dst, in0=src,
                                    scalar1=m[:, 0:1], scalar2=v,
                                    op0=mybir.AluOpType.subtract,
                                    op1=mybir.AluOpType.mult)
            nc.vector.tensor_copy(out=Ab, in_=dst)

        for blk in range(NB):
            # ---- Q, K projections ----
            for ki, kind in enumerate(("q", "k")):
                widx = WIDX[kind] * NB + blk
                for h in range(2):
                    ps = psS.tile([128, 4, 512], F32, tag="ps")
                    for u in range(4):
                        m4 = 4 * h + u
                        nc.tensor.matmul(ps[:, u, :], Wb[:, widx, :],
                                         Ab[:, 2 * m4:2 * m4 + 2, :],
                                         start=True, stop=True)
                    nc.vector.tensor_scalar(
                        out=QK[:, ki, 8 * h:8 * h + 8, :],
                        in0=ps.rearrange("p u (g j) -> p (u g) j", g=2),
                        scalar1=bcol[:, widx:widx + 1], scalar2=None,
                        op0=mybir.AluOpType.add)
            # ---- Qprep: block-diagonal q for paired scores (DMA: 16-part base) ----
            for s in range(8):
                sl = slice(16 * s, 16 * s + 16)
                nc.gpsimd.dma_start(
                    out=Qprep[sl, :, 256 * (s % 2):256 * (s % 2) + 256],
                    in_=QK[sl, 0, :, :])
            # ---- bv broadcast ----
            bvb = sml.tile([128, 256], F32, tag="bvb")
            nc.gpsimd.partition_broadcast(out_ap=bvb, in_ap=bvr[0:1, blk, :],
                                          channels=128)
            vwidx = WIDX["v"] * NB + blk

            for g in range(NG):
                # ---- vT: transposed V projection ----
                pv = psA.tile([128, 512], F32, tag="pa")
                for c in range(2):
                    nc.tensor.matmul(pv[:, 128 * c:128 * c + 128],
                                     Ab[:, g, 128 * c:128 * c + 128],
                                     Wb[:, vwidx, :], start=True, stop=True)
                nc.vector.tensor_tensor(
                    out=vTs[:, g, :, :].rearrange("p c q -> p (c q)"),
                    in0=pv[:, 0:256], in1=bvb, op=mybir.AluOpType.add)
                # ---- scores + exp ----
                ep = expp.tile([128, 4, 2, 512], BF16, tag="ep")
                for half in range(2):
                    sc = psS.tile([128, 4, 512], F32, tag="ps")
                    for q2 in range(2):
                        p4 = 2 * half + q2
                        pb = 32 * p4
                        for c in range(2):
                            nc.tensor.matmul(
                                sc[:, 2 * q2 + c, :],
                                QK[pb:pb + 32, 1, g, 128 * c:128 * c + 128],
                                Qprep[pb:pb + 32, g, :],
                                start=True, stop=True, tile_position=(pb, 0))
                    nc.scalar.activation(out=ep[:, 2 * half:2 * half + 2, :, :],
                                         in_=sc.rearrange("p (q c) j -> p q c j", q=2),
                                         func=mybir.ActivationFunctionType.Exp,
                                         scale=0.25)
                # ---- softmax denominators ----
                Dg = sml.tile([128, 4, 2, 512], BF16, tag="Dg")
                nc.gpsimd.partition_all_reduce(
                    out_ap=Dg, in_ap=ep, channels=128,
                    reduce_op=bass.bass_isa.ReduceOp.add)
                rr = sml.tile([128, 4, 512], F32, tag="rr")
                nc.vector.tensor_tensor(out=rr, in0=Dg[:, :, 0, :],
                                        in1=Dg[:, :, 1, :],
                                        op=mybir.AluOpType.add)
                nc.vector.reciprocal(out=rr, in_=rr)
                # ---- attention + normalize + residual ----
                for p4 in range(4):
                    pa = psA.tile([128, 512], F32, tag="pa")
                    nc.tensor.matmul(pa, vTs[:, g, 0, :], ep[:, p4, 0, :],
                                     start=True, stop=False)
                    nc.tensor.matmul(pa, vTs[:, g, 1, :], ep[:, p4, 1, :],
                                     start=False, stop=True)
                    an = sml.tile([128, 512], F32, tag="an")
                    b32 = slice(32 * p4, 32 * p4 + 32)
                    odd = slice(32 * p4 + 16, 32 * p4 + 32)
                    nc.vector.tensor_mul(out=an[b32, :], in0=pa[b32, :],
                                         in1=rr[b32, p4, :])
                    nc.gpsimd.dma_start(out=an[odd, 0:256], in_=an[odd, 256:512])
                    nc.gpsimd.tensor_tensor(
                        out=Y[b32, g, :], in0=an[b32, 0:256],
                        in1=A[b32, g, :], op=mybir.AluOpType.add)

            layernorm(Y, A)

            # ---- FFN ----
            w1i = WIDX["w1"] * NB + blk
            w2i = WIDX["w2"] * NB + blk
            for h in range(2):
                ps = psS.tile([128, 4, 512], F32, tag="ps")
                for u in range(4):
                    m4 = 4 * h + u
                    nc.tensor.matmul(ps[:, u, :], Wb[:, w1i, :],
                                     Ab[:, 2 * m4:2 * m4 + 2, :],
                                     start=True, stop=True)
                nc.scalar.activation(
                    out=H[:, 8 * h:8 * h + 8, :],
                    in_=ps.rearrange("p u (g j) -> p (u g) j", g=2),
                    func=mybir.ActivationFunctionType.Relu,
                    bias=bcol[:, w1i:w1i + 1], scale=1.0)
            for h in range(2):
                ps2 = psS.tile([128, 4, 512], F32, tag="ps")
                for u in range(4):
                    m4 = 4 * h + u
                    nc.tensor.matmul(ps2[:, u, :], Wb[:, w2i, :],
                                     H[:, 2 * m4:2 * m4 + 2, :],
                                     start=True, stop=True)
                ff = sml.tile([128, 8, S], F32, tag="ff")
                nc.vector.tensor_scalar(
                    out=ff, in0=ps2.rearrange("p u (g j) -> p (u g) j", g=2),
                    scalar1=bcol[:, w2i:w2i + 1], scalar2=None,
                    op0=mybir.AluOpType.add)
                nc.gpsimd.tensor_tensor(out=Y[:, 8 * h:8 * h + 8, :], in0=ff,
                                        in1=A[:, 8 * h:8 * h + 8, :],
                                        op=mybir.AluOpType.add)

            layernorm(Y, A)

        # ---- int8 quantized output: round(A * OSC), clamped ----
        qt = state.tile([128, NG, S], F32, tag="qt")
        nc.vector.tensor_scalar(out=qt, in0=A, scalar1=OSC, scalar2=None,
                                op0=mybir.AluOpType.mult)
        sg = state.tile([128, NG, S], F32, tag="sg")
        nc.scalar.activation(out=sg, in_=A,
                             func=mybir.ActivationFunctionType.Sign)
        nc.vector.tensor_scalar(out=sg, in0=sg, scalar1=0.5, scalar2=None,
                                op0=mybir.AluOpType.mult)
        nc.vector.tensor_tensor(out=qt, in0=qt, in1=sg,
                                op=mybir.AluOpType.add)
        nc.vector.tensor_scalar(out=qt, in0=qt, scalar1=127.0, scalar2=-127.0,
                                op0=mybir.AluOpType.min,
                                op1=mybir.AluOpType.max)
        Ob = state.tile([128, NG, S], mybir.dt.int8, tag="Ob")
        nc.vector.tensor_copy(out=Ob, in_=qt)
        nc.gpsimd.dma_start(out=out_d[:, :].rearrange("p (g j) -> p g j", g=NG),
                            in_=Ob)
        ctx.close()
    nc.finalize()
    return nc


def _host_prep(tokens, embed, Wq, bq, Wk, bk, Wv, bv, W1, b1, W2, b2):
    tokens = np.asarray(tokens)
    x0 = np.asarray(embed, np.float32)[tokens] + _make_pe()[None, :, :]
    Ws = {"q": Wq, "k": Wk, "v": Wv, "w1": W1, "w2": W2}
    Bs = {"q": bq, "k": bk, "v": bv, "w1": b1, "w2": b2}
    wd = np.zeros((10, D, D), np.float32)
    for kind, idx in WIDX.items():
        Wn = np.asarray(Ws[kind], np.float32)
        for blk in range(NB):
            wd[idx * NB + blk] = Wn[blk].T
    bc = np.zeros((128, 10), np.float32)
    for kind, idx in WIDX.items():
        bn = np.asarray(Bs[kind], np.float32)
        for blk in range(NB):
            bc[:, idx * NB + blk] = np.tile(bn[blk], 8)
    bvv = np.zeros((1, NB, 256), np.float32)
    for blk in range(NB):
        bvv[0, blk] = np.tile(np.asarray(Bs["v"], np.float32)[blk], 16)
    ins = []
    for core in range(NCORES):
        sh = x0[core * BS:(core + 1) * BS]                  # [128,S,D]
        xi = sh.reshape(NG, 8, S, D).transpose(1, 3, 0, 2)  # [8,D,NG,S]
        ins.append({
            "xq": np.ascontiguousarray(
                xi.reshape(128, NG * S)).astype(ml_dtypes.bfloat16),
            "wd": wd, "bc": bc, "bv": bvv,
        })
    return ins


def kernel(**inputs):
    if "nc" not in _CACHE:
        _CACHE["nc"] = _build_program()
    nc = _CACHE["nc"]
    in_maps = _host_prep(**inputs)
    res = run_bass_kernel_spmd(nc, in_maps, core_ids=list(range(NCORES)))
    outs = []
    for core in range(NCORES):
        o = np.asarray(res.results[core]["out"]).astype(np.float32) / OSC
        o = o.reshape(8, D, NG, S)
        outs.append(o.transpose(2, 0, 3, 1).reshape(BS, S, D))
    return np.concatenate(outs, axis=0).astype(np.float32)


# revision 5
# speedup vs baseline: 1.0976x; 1.0944x over previous
import sys, os
sys.path.insert(0, "/opt/trn_rl_repo")
import numpy as np
import ml_dtypes

import concourse.bass as bass
import concourse.bacc as bacc
import concourse.tile as tile
from concourse import mybir
from concourse.bass_utils import run_bass_kernel_spmd

B, S, D = 1024, 256, 16
NB = 2
NG = 16               # groups of 8 seqs; seq q = 8*g + sub
NCORES = 8
BS = B // NCORES      # 128 seqs per core
EPS = 1e-5
F32 = mybir.dt.float32
BF16 = mybir.dt.bfloat16

USE_CC = True         # global (exact) layernorm via AllReduce
OSC = 14.5            # int8 output quantization scale
_CACHE = {}

# weight kind -> index base in the packed [10,16,16] weight tensor
WIDX = {"q": 0, "k": 1, "v": 2, "w1": 3, "w2": 4}


def _make_pe():
    pos = np.arange(300)[:, None].astype(np.float32)
    div = np.exp(np.arange(0, D, 2).astype(np.float32) * (-np.log(10000.0) / D))
    pe = np.zeros((300, D), dtype=np.float32)
    pe[:, 0::2] = np.sin(pos * div)
    pe[:, 1::2] = np.cos(pos * div)
    return pe[:S]


def _build_program():
    nc = bacc.Bacc(num_devices=NCORES)
    xq_d = nc.dram_tensor("xq", [128, NG * S], BF16, kind="ExternalInput")
    wd_d = nc.dram_tensor("wd", [10, D, D], F32, kind="ExternalInput")
    bc_d = nc.dram_tensor("bc", [128, 10], F32, kind="ExternalInput")
    bv_d = nc.dram_tensor("bv", [1, NB, 256], F32, kind="ExternalInput")
    out_d = nc.dram_tensor("out", [128, NG * S], mybir.dt.int8,
                           kind="ExternalOutput")
    if USE_CC:
        ccd = nc.dram_tensor("ccd", [128, 2], F32, kind="Internal")
        cco = nc.dram_tensor("cco", [128, 2], F32, kind="Internal",
                             addr_space="Shared")

    with tile.TileContext(nc) as tc:
        from contextlib import ExitStack
        ctx = ExitStack()
        consts = ctx.enter_context(tc.tile_pool(name="consts", bufs=1))
        state = ctx.enter_context(tc.tile_pool(name="state", bufs=1))
        expp = ctx.enter_context(tc.tile_pool(name="expp", bufs=2))
        sml = ctx.enter_context(tc.tile_pool(name="sml", bufs=2))
        psS = ctx.enter_context(tc.tile_pool(name="psS", bufs=1, space="PSUM"))
        psA = ctx.enter_context(tc.tile_pool(name="psA", bufs=3, space="PSUM"))

        # ---- constants ----
        Wall = consts.tile([128, 10, 128], F32, tag="Wall")
        nc.vector.memset(Wall, 0.0)
        for s in range(8):
            nc.gpsimd.dma_start(
                out=Wall[16 * s:16 * s + 16, :, 16 * s:16 * s + 16],
                in_=wd_d.rearrange("w a b -> a w b"))
        Wb = consts.tile([128, 10, 128], BF16, tag="Wb")
        nc.vector.tensor_copy(out=Wb, in_=Wall)
        bcol = consts.tile([128, 10], F32, tag="bcol")
        nc.gpsimd.dma_start(out=bcol, in_=bc_d[:, :])
        bvr = consts.tile([1, NB, 256], F32, tag="bvr")
        nc.gpsimd.dma_start(out=bvr, in_=bv_d[:, :, :])
        eps_t = consts.tile([128, 1], F32, tag="eps")
        nc.vector.memset(eps_t, EPS)
        Qprep = consts.tile([128, NG, 512], BF16, tag="Qprep")
        nc.vector.memset(Qprep, 0.0)

        # ---- state ----
        A = state.tile([128, NG, S], F32, tag="A")
        Ab = state.tile([128, NG, S], BF16, tag="Ab")
        Y = state.tile([128, NG, S], F32, tag="Y")
        QK = state.tile([128, 2, NG, S], BF16, tag="QK")
        vTs = state.tile([128, NG, 2, 128], BF16, tag="vTs")
        H = state.tile([128, NG, S], BF16, tag="H")

        nc.gpsimd.dma_start(out=Ab, in_=xq_d[:, :].rearrange("p (g j) -> p g j", g=NG))
        nc.vector.tensor_copy(out=A, in_=Ab)

        def layernorm(src, dst):
            st = sml.tile([128, 2], F32, tag="st")
            nc.vector.tensor_reduce(out=st[:, 0:1], in_=src,
                                    axis=mybir.AxisListType.XY,
                                    op=mybir.AluOpType.add)
            nc.scalar.activation(out=H, in_=src,
                                 func=mybir.ActivationFunctionType.Square,
                                 accum_out=st[:, 1:2])
            gs = sml.tile([128, 2], F32, tag="gs")
            nc.gpsimd.partition_all_reduce(out_ap=gs, in_ap=st, channels=128,
                                           reduce_op=bass.bass_isa.ReduceOp.add)
            n_tot = 128 * NG * S * (NCORES if USE_CC else 1)
            if USE_CC:
                nc.gpsimd.dma_start(out=ccd[:, :], in_=gs)
                nc.gpsimd.collective_compute(
                    kind="AllReduce", op=mybir.AluOpType.add,
                    replica_groups=[list(range(NCORES))],
                    ins=[ccd[:, :]], outs=[cco[:, :]])
                nc.gpsimd.dma_start(out=gs, in_=cco[:, :])
            m = sml.tile([128, 2], F32, tag="m")
            nc.vector.tensor_scalar(out=m, in0=gs, scalar1=1.0 / n_tot,
                                    scalar2=None, op0=mybir.AluOpType.mult)
            v = sml.tile([128, 1], F32, tag="v")
            nc.vector.tensor_mul(out=v, in0=m[:, 0:1], in1=m[:, 0:1])
            nc.vector.tensor_tensor(out=v, in0=m[:, 1:2], in1=v,
                                    op=mybir.AluOpType.subtract)
            nc.scalar.activation(out=v, in_=v,
                                 func=mybir.ActivationFunctionType.Sqrt,
                                 bias=eps_t, scale=1.0)
            nc.vector.reciprocal(out=v, in_=v)
            nc.vector.tensor_scalar(out=dst, in0=src,
                                    scalar1=m[:, 0:1], scalar2=v,
                                    op0=mybir.AluOpType.subtract,
                                    op1=mybir.AluOpType.mult)
            nc.vector.tensor_copy(out=Ab, in_=dst)

        for blk in range(NB):
            # ---- Q, K projections ----
            for ki, kind in enumerate(("q", "k")):
                widx = WIDX[kind] * NB + blk
                for h in range(2):
                    ps = psS.tile([128, 4, 512], F32, tag="ps")
                    for u in range(4):
                        m4 = 4 * h + u
                        nc.tensor.matmul(ps[:, u, :], Wb[:, widx, :],
                                         Ab[:, 2 * m4:2 * m4 + 2, :],
                                         start=True, stop=True)
                    nc.vector.tensor_scalar(
                        out=QK[:, ki, 8 * h:8 * h + 8, :],
                        in0=ps.rearrange("p u (g j) -> p (u g) j", g=2),
                        scalar1=bcol[:, widx:widx + 1], scalar2=None,
                        op0=mybir.AluOpType.add)
            # ---- Qprep: block-diagonal q for paired scores (DMA: 16-part base) ----
            for s in range(8):
                sl = slice(16 * s, 16 * s + 16)
                nc.scalar.dma_start(
                    out=Qprep[sl, :, 256 * (s % 2):256 * (s % 2) + 256],
                    in_=QK[sl, 0, :, :])
            # ---- bv broadcast ----
            bvb = sml.tile([128, 256], F32, tag="bvb")
            nc.gpsimd.partition_broadcast(out_ap=bvb, in_ap=bvr[0:1, blk, :],
                                          channels=128)
            vwidx = WIDX["v"] * NB + blk

            for g in range(NG):
                # ---- vT: transposed V projection ----
                pv = psA.tile([128, 512], F32, tag="pa")
                for c in range(2):
                    nc.tensor.matmul(pv[:, 128 * c:128 * c + 128],
                                     Ab[:, g, 128 * c:128 * c + 128],
                                     Wb[:, vwidx, :], start=True, stop=True)
                nc.vector.tensor_tensor(
                    out=vTs[:, g, :, :].rearrange("p c q -> p (c q)"),
                    in0=pv[:, 0:256], in1=bvb, op=mybir.AluOpType.add)
                # ---- scores + exp ----
                ep = expp.tile([128, 4, 2, 512], BF16, tag="ep")
                for half in range(2):
                    sc = psS.tile([128, 4, 512], F32, tag="ps")
                    for q2 in range(2):
                        p4 = 2 * half + q2
                        pb = 32 * p4
                        for c in range(2):
                            nc.tensor.matmul(
                                sc[:, 2 * q2 + c, :],
                                QK[pb:pb + 32, 1, g, 128 * c:128 * c + 128],
                                Qprep[pb:pb + 32, g, :],
                                start=True, stop=True, tile_position=(pb, 0))
                    nc.scalar.activation(out=ep[:, 2 * half:2 * half + 2, :, :],
                                         in_=sc.rearrange("p (q c) j -> p q c j", q=2),
                                         func=mybir.ActivationFunctionType.Exp,
                                         scale=0.25)
                # ---- softmax denominators ----
                Dg = sml.tile([128, 4, 2, 512], BF16, tag="Dg")
                nc.gpsimd.partition_all_reduce(
                    out_ap=Dg, in_ap=ep, channels=128,
                    reduce_op=bass.bass_isa.ReduceOp.add)
                rr = sml.tile([128, 4, 512], F32, tag="rr")
                nc.vector.tensor_tensor(out=rr, in0=Dg[:, :, 0, :],
                                        in1=Dg[:, :, 1, :],
                                        op=mybir.AluOpType.add)
                nc.vector.reciprocal(out=rr, in_=rr)
                # ---- attention + normalize + residual ----
                for p4 in range(4):
                    pa = psA.tile([128, 512], F32, tag="pa")
                    nc.tensor.matmul(pa, vTs[:, g, 0, :], ep[:, p4, 0, :],
                                     start=True, stop=False)
                    nc.tensor.matmul(pa, vTs[:, g, 1, :], ep[:, p4, 1, :],
                                     start=False, stop=True)
                    an = sml.tile([128, 512], F32, tag="an")
                    b32 = slice(32 * p4, 32 * p4 + 32)
                    odd = slice(32 * p4 + 16, 32 * p4 + 32)
                    nc.vector.tensor_mul(out=an[b32, :], in0=pa[b32, :],
                                         in1=rr[b32, p4, :])
                    nc.sync.dma_start(out=an[odd, 0:256], in_=an[odd, 256:512])
                    nc.gpsimd.tensor_tensor(
                        out=Y[b32, g, :], in0=an[b32, 0:256],
                        in1=A[b32, g, :], op=mybir.AluOpType.add)

            layernorm(Y, A)

            # ---- FFN ----
            w1i = WIDX["w1"] * NB + blk
            w2i = WIDX["w2"] * NB + blk
            for h in range(2):
                ps = psS.tile([128, 4, 512], F32, tag="ps")
                for u in range(4):
                    m4 = 4 * h + u
                    nc.tensor.matmul(ps[:, u, :], Wb[:, w1i, :],
                                     Ab[:, 2 * m4:2 * m4 + 2, :],
                                     start=True, stop=True)
                nc.scalar.activation(
                    out=H[:, 8 * h:8 * h + 8, :],
                    in_=ps.rearrange("p u (g j) -> p (u g) j", g=2),
                    func=mybir.ActivationFunctionType.Relu,
                    bias=bcol[:, w1i:w1i + 1], scale=1.0)
            for h in range(2):
                ps2 = psS.tile([128, 4, 512], F32, tag="ps")
                for u in range(4):
                    m4 = 4 * h + u
                    nc.tensor.matmul(ps2[:, u, :], Wb[:, w2i, :],
                                     H[:, 2 * m4:2 * m4 + 2, :],
                                     start=True, stop=True)
                ff = sml.tile([128, 8, S], F32, tag="ff")
                nc.vector.tensor_scalar(
                    out=ff, in0=ps2.rearrange("p u (g j) -> p (u g) j", g=2),
                    scalar1=bcol[:, w2i:w2i + 1], scalar2=None,
                    op0=mybir.AluOpType.add)
                nc.gpsimd.tensor_tensor(out=Y[:, 8 * h:8 * h + 8, :], in0=ff,
                                        in1=A[:, 8 * h:8 * h + 8, :],
                                        op=mybir.AluOpType.add)

            layernorm(Y, A)

        # ---- int8 quantized output: round(A * OSC), clamped ----
        qt = state.tile([128, NG, S], F32, tag="qt")
        nc.vector.tensor_scalar(out=qt, in0=A, scalar1=OSC, scalar2=None,
                                op0=mybir.AluOpType.mult)
        sg = state.tile([128, NG, S], F32, tag="sg")
        nc.scalar.activation(out=sg, in_=A,
                             func=mybir.ActivationFunctionType.Sign)
        nc.vector.tensor_scalar(out=sg, in0=sg, scalar1=0.5, scalar2=None,
                                op0=mybir.AluOpType.mult)
        nc.vector.tensor_tensor(out=qt, in0=qt, in1=sg,
                                op=mybir.AluOpType.add)
        nc.vector.tensor_scalar(out=qt, in0=qt, scalar1=127.0, scalar2=-127.0,
                                op0=mybir.AluOpType.min,
                                op1=mybir.AluOpType.max)
        Ob = state.tile([128, NG, S], mybir.dt.int8, tag="Ob")
        nc.vector.tensor_copy(out=Ob, in_=qt)
        nc.gpsimd.dma_start(out=out_d[:, :].rearrange("p (g j) -> p g j", g=NG),
                            in_=Ob)
        ctx.close()
    nc.finalize()
    return nc


def _host_prep(tokens, embed, Wq, bq, Wk, bk, Wv, bv, W1, b1, W2, b2):
    tokens = np.asarray(tokens)
    x0 = np.asarray(embed, np.float32)[tokens] + _make_pe()[None, :, :]
    Ws = {"q": Wq, "k": Wk, "v": Wv, "w1": W1, "w2": W2}
    Bs = {"q": bq, "k": bk, "v": bv, "w1": b1, "w2": b2}
    wd = np.zeros((10, D, D), np.float32)
    for kind, idx in WIDX.items():
        Wn = np.asarray(Ws[kind], np.float32)
        for blk in range(NB):
            wd[idx * NB + blk] = Wn[blk].T
    bc = np.zeros((128, 10), np.float32)
    for kind, idx in WIDX.items():
        bn = np.asarray(Bs[kind], np.float32)
        for blk in range(NB):
            bc[:, idx * NB + blk] = np.tile(bn[blk], 8)
    bvv = np.zeros((1, NB, 256), np.float32)
    for blk in range(NB):
        bvv[0, blk] = np.tile(np.asarray(Bs["v"], np.float32)[blk], 16)
    ins = []
    for core in range(NCORES):
        sh = x0[core * BS:(core + 1) * BS]                  # [128,S,D]
        xi = sh.reshape(NG, 8, S, D).transpose(1, 3, 0, 2)  # [8,D,NG,S]
        ins.append({
            "xq": np.ascontiguousarray(
                xi.reshape(128, NG * S)).astype(ml_dtypes.bfloat16),
            "wd": wd, "bc": bc, "bv": bvv,
        })
    return ins


def kernel(**inputs):
    if "nc" not in _CACHE:
        _CACHE["nc"] = _build_program()
    nc = _CACHE["nc"]
    in_maps = _host_prep(**inputs)
    res = run_bass_kernel_spmd(nc, in_maps, core_ids=list(range(NCORES)))
    outs = []
    for core in range(NCORES):
        o = np.asarray(res.results[core]["out"]).astype(np.float32) / OSC
        o = o.reshape(8, D, NG, S)
        outs.append(o.transpose(2, 0, 3, 1).reshape(BS, S, D))
    return np.concatenate(outs, axis=0).astype(np.float32)


# revision 6
# speedup vs baseline: 1.1020x; 1.0040x over previous
import sys, os
sys.path.insert(0, "/opt/trn_rl_repo")
import numpy as np
import ml_dtypes

import concourse.bass as bass
import concourse.bacc as bacc
import concourse.tile as tile
from concourse import mybir
from concourse.bass_utils import run_bass_kernel_spmd

B, S, D = 1024, 256, 16
NB = 2
NG = 16               # groups of 8 seqs; seq q = 8*g + sub
NCORES = 8
BS = B // NCORES      # 128 seqs per core
EPS = 1e-5
F32 = mybir.dt.float32
BF16 = mybir.dt.bfloat16

USE_CC = True         # global (exact) layernorm via AllReduce
OSC = 14.5            # int8 output quantization scale
_CACHE = {}

# weight kind -> index base in the packed [10,16,16] weight tensor
WIDX = {"q": 0, "k": 1, "v": 2, "w1": 3, "w2": 4}


def _make_pe():
    pos = np.arange(300)[:, None].astype(np.float32)
    div = np.exp(np.arange(0, D, 2).astype(np.float32) * (-np.log(10000.0) / D))
    pe = np.zeros((300, D), dtype=np.float32)
    pe[:, 0::2] = np.sin(pos * div)
    pe[:, 1::2] = np.cos(pos * div)
    return pe[:S]


def _build_program():
    nc = bacc.Bacc(num_devices=NCORES)
    xq_d = nc.dram_tensor("xq", [128, NG * S], BF16, kind="ExternalInput")
    wd_d = nc.dram_tensor("wd", [10, D, D], F32, kind="ExternalInput")
    bc_d = nc.dram_tensor("bc", [128, 10], F32, kind="ExternalInput")
    bv_d = nc.dram_tensor("bv", [1, NB, 256], F32, kind="ExternalInput")
    out_d = nc.dram_tensor("out", [128, NG * S], mybir.dt.int8,
                           kind="ExternalOutput")
    if USE_CC:
        ccd = nc.dram_tensor("ccd", [128, 2], F32, kind="Internal")
        cco = nc.dram_tensor("cco", [128, 2], F32, kind="Internal",
                             addr_space="Shared")

    with tile.TileContext(nc) as tc:
        from contextlib import ExitStack
        ctx = ExitStack()
        consts = ctx.enter_context(tc.tile_pool(name="consts", bufs=1))
        state = ctx.enter_context(tc.tile_pool(name="state", bufs=1))
        expp = ctx.enter_context(tc.tile_pool(name="expp", bufs=2))
        sml = ctx.enter_context(tc.tile_pool(name="sml", bufs=2))
        psS = ctx.enter_context(tc.tile_pool(name="psS", bufs=1, space="PSUM"))
        psA = ctx.enter_context(tc.tile_pool(name="psA", bufs=3, space="PSUM"))

        # ---- constants ----
        Wall = consts.tile([128, 10, 128], F32, tag="Wall")
        nc.vector.memset(Wall, 0.0)
        for s in range(8):
            nc.gpsimd.dma_start(
                out=Wall[16 * s:16 * s + 16, :, 16 * s:16 * s + 16],
                in_=wd_d.rearrange("w a b -> a w b"))
        Wb = consts.tile([128, 10, 128], BF16, tag="Wb")
        nc.vector.tensor_copy(out=Wb, in_=Wall)
        bcol = consts.tile([128, 10], F32, tag="bcol")
        nc.gpsimd.dma_start(out=bcol, in_=bc_d[:, :])
        bvr = consts.tile([1, NB, 256], F32, tag="bvr")
        nc.gpsimd.dma_start(out=bvr, in_=bv_d[:, :, :])
        eps_t = consts.tile([128, 1], F32, tag="eps")
        nc.vector.memset(eps_t, EPS)
        Qprep = consts.tile([128, NG, 512], BF16, tag="Qprep")
        nc.vector.memset(Qprep, 0.0)

        # ---- state ----
        A = state.tile([128, NG, S], F32, tag="A")
        Ab = state.tile([128, NG, S], BF16, tag="Ab")
        Y = state.tile([128, NG, S], F32, tag="Y")
        QK = state.tile([128, 2, NG, S], BF16, tag="QK")
        vTs = state.tile([128, NG, 2, 128], BF16, tag="vTs")
        H = state.tile([128, NG, S], BF16, tag="H")

        nc.gpsimd.dma_start(out=Ab, in_=xq_d[:, :].rearrange("p (g j) -> p g j", g=NG))
        nc.vector.tensor_copy(out=A, in_=Ab)

        def layernorm(src, dst):
            st = sml.tile([128, 2], F32, tag="st")
            nc.vector.tensor_reduce(out=st[:, 0:1], in_=src,
                                    axis=mybir.AxisListType.XY,
                                    op=mybir.AluOpType.add)
            nc.scalar.activation(out=H, in_=src,
                                 func=mybir.ActivationFunctionType.Square,
                                 accum_out=st[:, 1:2])
            gs = sml.tile([128, 2], F32, tag="gs")
            nc.gpsimd.partition_all_reduce(out_ap=gs, in_ap=st, channels=128,
                                           reduce_op=bass.bass_isa.ReduceOp.add)
            n_tot = 128 * NG * S * (NCORES if USE_CC else 1)
            if USE_CC:
                nc.gpsimd.dma_start(out=ccd[:, :], in_=gs)
                nc.gpsimd.collective_compute(
                    kind="AllReduce", op=mybir.AluOpType.add,
                    replica_groups=[list(range(NCORES))],
                    ins=[ccd[:, :]], outs=[cco[:, :]])
                nc.gpsimd.dma_start(out=gs, in_=cco[:, :])
            m = sml.tile([128, 2], F32, tag="m")
            nc.vector.tensor_scalar(out=m, in0=gs, scalar1=1.0 / n_tot,
                                    scalar2=None, op0=mybir.AluOpType.mult)
            v = sml.tile([128, 1], F32, tag="v")
            nc.vector.tensor_mul(out=v, in0=m[:, 0:1], in1=m[:, 0:1])
            nc.vector.tensor_tensor(out=v, in0=m[:, 1:2], in1=v,
                                    op=mybir.AluOpType.subtract)
            nc.scalar.activation(out=v, in_=v,
                                 func=mybir.ActivationFunctionType.Sqrt,
                                 bias=eps_t, scale=1.0)
            nc.vector.reciprocal(out=v, in_=v)
            nc.vector.tensor_scalar(out=dst, in0=src,
                                    scalar1=m[:, 0:1], scalar2=v,
                                    op0=mybir.AluOpType.subtract,
                                    op1=mybir.AluOpType.mult)
            nc.vector.tensor_copy(out=Ab, in_=dst)

        for blk in range(NB):
            # ---- Q, K projections ----
            for ki, kind in enumerate(("q", "k")):
                widx = WIDX[kind] * NB + blk
                for h in range(2):
                    ps = psS.tile([128, 4, 512], F32, tag="ps")
                    for u in range(4):
                        m4 = 4 * h + u
                        nc.tensor.matmul(ps[:, u, :], Wb[:, widx, :],
                                         Ab[:, 2 * m4:2 * m4 + 2, :],
                                         start=True, stop=True)
                    nc.vector.tensor_scalar(
                        out=QK[:, ki, 8 * h:8 * h + 8, :],
                        in0=ps.rearrange("p u (g j) -> p (u g) j", g=2),
                        scalar1=bcol[:, widx:widx + 1], scalar2=None,
                        op0=mybir.AluOpType.add)
            # ---- Qprep: block-diagonal q for paired scores (DMA: 16-part base) ----
            for s in range(8):
                sl = slice(16 * s, 16 * s + 16)
                nc.scalar.dma_start(
                    out=Qprep[sl, :, 256 * (s % 2):256 * (s % 2) + 256],
                    in_=QK[sl, 0, :, :])
            # ---- bv broadcast ----
            bvb = sml.tile([128, 256], F32, tag="bvb")
            nc.gpsimd.partition_broadcast(out_ap=bvb, in_ap=bvr[0:1, blk, :],
                                          channels=128)
            vwidx = WIDX["v"] * NB + blk

            for g in range(NG):
                # ---- vT: transposed V projection ----
                pv = psA.tile([128, 512], F32, tag="pa")
                for c in range(2):
                    nc.tensor.matmul(pv[:, 128 * c:128 * c + 128],
                                     Ab[:, g, 128 * c:128 * c + 128],
                                     Wb[:, vwidx, :], start=True, stop=True)
                nc.vector.tensor_tensor(
                    out=vTs[:, g, :, :].rearrange("p c q -> p (c q)"),
                    in0=pv[:, 0:256], in1=bvb, op=mybir.AluOpType.add)
                # ---- scores + exp ----
                ep = expp.tile([128, 4, 2, 512], BF16, tag="ep")
                for half in range(2):
                    sc = psS.tile([128, 4, 512], F32, tag="ps")
                    for q2 in range(2):
                        p4 = 2 * half + q2
                        pb = 32 * p4
                        for c in range(2):
                            nc.tensor.matmul(
                                sc[:, 2 * q2 + c, :],
                                QK[pb:pb + 32, 1, g, 128 * c:128 * c + 128],
                                Qprep[pb:pb + 32, g, :],
                                start=True, stop=True, tile_position=(pb, 0))
                    nc.scalar.activation(out=ep[:, 2 * half:2 * half + 2, :, :],
                                         in_=sc.rearrange("p (q c) j -> p q c j", q=2),
                                         func=mybir.ActivationFunctionType.Exp,
                                         scale=0.25)
                # ---- softmax denominators ----
                Dg = sml.tile([128, 4, 2, 512], BF16, tag="Dg")
                nc.gpsimd.partition_all_reduce(
                    out_ap=Dg, in_ap=ep, channels=128,
                    reduce_op=bass.bass_isa.ReduceOp.add)
                rr = sml.tile([128, 4, 512], F32, tag="rr")
                nc.gpsimd.tensor_tensor(out=rr, in0=Dg[:, :, 0, :],
                                        in1=Dg[:, :, 1, :],
                                        op=mybir.AluOpType.add)
                nc.vector.reciprocal(out=rr, in_=rr)
                # ---- attention + normalize + residual ----
                for p4 in range(4):
                    pa = psA.tile([128, 512], F32, tag="pa")
                    nc.tensor.matmul(pa, vTs[:, g, 0, :], ep[:, p4, 0, :],
                                     start=True, stop=False)
                    nc.tensor.matmul(pa, vTs[:, g, 1, :], ep[:, p4, 1, :],
                                     start=False, stop=True)
                    an = sml.tile([128, 512], F32, tag="an")
                    b32 = slice(32 * p4, 32 * p4 + 32)
                    odd = slice(32 * p4 + 16, 32 * p4 + 32)
                    nc.vector.tensor_mul(out=an[b32, :], in0=pa[b32, :],
                                         in1=rr[b32, p4, :])
                    nc.sync.dma_start(out=an[odd, 0:256], in_=an[odd, 256:512])
                    nc.gpsimd.tensor_tensor(
                        out=Y[b32, g, :], in0=an[b32, 0:256],
                        in1=A[b32, g, :], op=mybir.AluOpType.add)

            layernorm(Y, A)

            # ---- FFN ----
            w1i = WIDX["w1"] * NB + blk
            w2i = WIDX["w2"] * NB + blk
            for h in range(2):
                ps = psS.tile([128, 4, 512], F32, tag="ps")
                for u in range(4):
                    m4 = 4 * h + u
                    nc.tensor.matmul(ps[:, u, :], Wb[:, w1i, :],
                                     Ab[:, 2 * m4:2 * m4 + 2, :],
                                     start=True, stop=True)
                nc.scalar.activation(
                    out=H[:, 8 * h:8 * h + 8, :],
                    in_=ps.rearrange("p u (g j) -> p (u g) j", g=2),
                    func=mybir.ActivationFunctionType.Relu,
                    bias=bcol[:, w1i:w1i + 1], scale=1.0)
            for h in range(2):
                ps2 = psS.tile([128, 4, 512], F32, tag="ps")
                for u in range(4):
                    m4 = 4 * h + u
                    nc.tensor.matmul(ps2[:, u, :], Wb[:, w2i, :],
                                     H[:, 2 * m4:2 * m4 + 2, :],
                                     start=True, stop=True)
                ff = sml.tile([128, 8, S], F32, tag="ff")
                nc.vector.tensor_scalar(
                    out=ff, in0=ps2.rearrange("p u (g j) -> p (u g) j", g=2),
                    scalar1=bcol[:, w2i:w2i + 1], scalar2=None,
                    op0=mybir.AluOpType.add)
                nc.gpsimd.tensor_tensor(out=Y[:, 8 * h:8 * h + 8, :], in0=ff,
                                        in1=A[:, 8 * h:8 * h + 8, :],
                                        op=mybir.AluOpType.add)

            layernorm(Y, A)

        # ---- int8 quantized output: round(A * OSC), clamped ----
        qt = state.tile([128, NG, S], F32, tag="qt")
        nc.vector.tensor_scalar(out=qt, in0=A, scalar1=OSC, scalar2=None,
                                op0=mybir.AluOpType.mult)
        sg = state.tile([128, NG, S], F32, tag="sg")
        nc.scalar.activation(out=sg, in_=A,
                             func=mybir.ActivationFunctionType.Sign)
        nc.vector.tensor_scalar(out=sg, in0=sg, scalar1=0.5, scalar2=None,
                                op0=mybir.AluOpType.mult)
        nc.vector.tensor_tensor(out=qt, in0=qt, in1=sg,
                                op=mybir.AluOpType.add)
        nc.vector.tensor_scalar(out=qt, in0=qt, scalar1=127.0, scalar2=-127.0,
                                op0=mybir.AluOpType.min,
                                op1=mybir.AluOpType.max)
        Ob = state.tile([128, NG, S], mybir.dt.int8, tag="Ob")
        nc.vector.tensor_copy(out=Ob, in_=qt)
        nc.gpsimd.dma_start(out=out_d[:, :].rearrange("p (g j) -> p g j", g=NG),
                            in_=Ob)
        ctx.close()
    nc.finalize()
    return nc


def _host_prep(tokens, embed, Wq, bq, Wk, bk, Wv, bv, W1, b1, W2, b2):
    tokens = np.asarray(tokens)
    x0 = np.asarray(embed, np.float32)[tokens] + _make_pe()[None, :, :]
    Ws = {"q": Wq, "k": Wk, "v": Wv, "w1": W1, "w2": W2}
    Bs = {"q": bq, "k": bk, "v": bv, "w1": b1, "w2": b2}
    wd = np.zeros((10, D, D), np.float32)
    for kind, idx in WIDX.items():
        Wn = np.asarray(Ws[kind], np.float32)
        for blk in range(NB):
            wd[idx * NB + blk] = Wn[blk].T
    bc = np.zeros((128, 10), np.float32)
    for kind, idx in WIDX.items():
        bn = np.asarray(Bs[kind], np.float32)
        for blk in range(NB):
            bc[:, idx * NB + blk] = np.tile(bn[blk], 8)
    bvv = np.zeros((1, NB, 256), np.float32)
    for blk in range(NB):
        bvv[0, blk] = np.tile(np.asarray(Bs["v"], np.float32)[blk], 16)
    ins = []
    for core in range(NCORES):
        sh = x0[core * BS:(core + 1) * BS]                  # [128,S,D]
        xi = sh.reshape(NG, 8, S, D).transpose(1, 3, 0, 2)  # [8,D,NG,S]
        ins.append({
            "xq": np.ascontiguousarray(
                xi.reshape(128, NG * S)).astype(ml_dtypes.bfloat16),
            "wd": wd, "bc": bc, "bv": bvv,
        })
    return ins


def kernel(**inputs):
    if "nc" not in _CACHE:
        _CACHE["nc"] = _build_program()
    nc = _CACHE["nc"]
    in_maps = _host_prep(**inputs)
    res = run_bass_kernel_spmd(nc, in_maps, core_ids=list(range(NCORES)))
    outs = []
    for core in range(NCORES):
        o = np.asarray(res.results[core]["out"]).astype(np.float32) / OSC
        o = o.reshape(8, D, NG, S)
        outs.append(o.transpose(2, 0, 3, 1).reshape(BS, S, D))
    return np.concatenate(outs, axis=0).astype(np.float32)
